# revision 1
# baseline (speedup 1.0000x reference)
"""Causal self-attention (B=4, T=1024, C=1024, H=16) on 8 Trainium2 cores.

Sharding: hybrid batch x head-group: core c owns batch c//2 and head-group
c%2 (8 heads = 4 pairs of 2). Each core computes its batch's qkv slice,
causal attention for its 8 heads, and a partial projection (contraction
over its 512 rows of w_proj). Host sums the 2 partials per batch + b_proj.

Design (~1.9x vs the 173.7us fp32r baseline; PE-saturated):
  - all matmul operands bf16 (host-converted inputs; PSUM accum stays f32).
    Removes fp32r N<256 4x penalties, halves DMA, same 1 cycle/row PE rate.
  - merged A|B PSUM tiles [128,2,512]: ONE exp per (qc,kb) step via a
    strided AP (halves Act-engine instruction count).
  - V transposed via plain matmul (lhsT=vT block, rhs=bf16 identity) into
    the shared "mm" PSUM tag; single strided DVE copy into V2e, whose 65th
    column of ones makes the PV matmul also accumulate softmax denominators.
  - global filler queue: QKV matmul chains of the NEXT pair and projection
    chains are interleaved between attention steps in the in-order PE
    queue, so the PE never idles waiting on exp -> PV dependencies (this
    also keeps the PE p-state ramped at 2.4 GHz on hardware; stalls
    otherwise drop it to 1.2 GHz).
  - normalization: reciprocal of the denominator row on DVE; O rows moved
    off PSUM early (Act/DVE split) to release the os bank for the next
    query chunk; 1/denom broadcast to 64 partitions by a DRAM
    round-trip DMA with a stride-0 source, multiplied in on DVE;
  - causal masking of the 8 diagonal blocks per pair: the P tile is zeroed
    above the diagonal in-place on the DVE (bf16 2x mode) after the exp,
    instead of -30000 PE matmul adds; the P-V matmul runs two steps behind
    its exp so the extra DVE hop fits in the pipeline slack;
  - partial projection output stored bf16 (halves output DMA); host
    accumulates the two partials per batch in f32.
"""

import collections

import numpy as np
import concourse.bass as bass
import concourse.mybir as mybir
import concourse.tile as tile
from concourse.bass import ts
from concourse.bass_utils import run_bass_kernel_spmd

F32 = mybir.dt.float32
F32R = mybir.dt.float32r
BF16 = mybir.dt.bfloat16
AF = mybir.ActivationFunctionType

B, T, C, H = 4, 1024, 1024, 16
D = C // H            # 64
NCORES = 8
NPAIR = 4             # head pairs per core
CT = C // 128         # 8 contraction tiles
KB = T // 128         # 8 key blocks
QC = T // 512         # 2 query chunks
NEG = -30000.0

BCAST_DMA = True
DMAT = False

_nc_cache = {}


def _split_sync_waits(nc):
    """This walrus build accepts exactly one sem-wait per instruction; move
    overflow waits onto fresh same-engine NoOps inserted just before."""
    n = 0
    for fn in nc.m.functions:
        for blk in fn.blocks:
            new_insts = []
            for inst in blk.instructions:
                si = getattr(inst, "sync_info", None)
                waits = list(si.on_wait) if si is not None and si.on_wait else []
                if len(waits) > 1:
                    for w in waits[1:]:
                        n += 1
                        new_insts.append(mybir.InstNoOp(
                            name=f"waitfix-{n}-{id(inst) & 0xffff}",
                            sync_info=mybir.SyncInfo(on_wait=[w], on_update=[]),
                            bass_nofuse=True,
                            engine=inst.engine,
                        ))
                    si.on_wait = waits[:1]
                new_insts.append(inst)
            blk.instructions[:] = new_insts
    return n


def build_nc(reps=1):
    nc = bass.Bass()
    xT_d = nc.dram_tensor("xTb", [C, T], BF16, kind="ExternalInput")
    wqkv_d = nc.dram_tensor("wqkv", [C, NPAIR * 3 * 128], BF16, kind="ExternalInput")
    battn_d = nc.dram_tensor("battn", [128, NPAIR * 3], F32, kind="ExternalInput")
    wproj_d = nc.dram_tensor("wproj", [NPAIR * 128, C], BF16, kind="ExternalInput")
    idb_d = nc.dram_tensor("idb", [128, 128], BF16, kind="ExternalInput")
    mask_d = nc.dram_tensor("mask", [128, 128], BF16, kind="ExternalInput")
    tri01_d = nc.dram_tensor("tri01", [128, 128], BF16, kind="ExternalInput")
    onesrow_d = nc.dram_tensor("onesrow", [1, 64], F32R, kind="ExternalInput")
    out_d = nc.dram_tensor("partial", [T, C], BF16, kind="ExternalOutput")
    bscr_d = [nc.dram_tensor(f"bscr{i}", [2, 512], F32R, kind="Internal")
              for i in range(2)]

    with tile.TileContext(nc) as tc:
        with tc.tile_pool(name="const", bufs=1) as cpool, \
             tc.tile_pool(name="wp", bufs=1) as wpool, \
             tc.tile_pool(name="xp", bufs=2) as xpool, \
             tc.tile_pool(name="qk", bufs=2) as qkpool, \
             tc.tile_pool(name="pp", bufs=4) as ppool, \
             tc.tile_pool(name="yp", bufs=5) as ypool, \
             tc.tile_pool(name="rp", bufs=2) as rpool, \
             tc.tile_pool(name="op", bufs=4) as opool, \
             tc.tile_pool(name="ps", bufs=1, space="PSUM") as ps:

            # ---- constants & weights ----
            idb = cpool.tile([128, 128], BF16)
            mask = cpool.tile([128, 128], BF16)
            battn = cpool.tile([128, NPAIR * 3], F32)
            tri01 = cpool.tile([128, 128], BF16)
            onesrow = cpool.tile([1, 64], F32R)
            nc.gpsimd.dma_start(out=idb, in_=idb_d.ap())
            nc.gpsimd.dma_start(out=mask, in_=mask_d.ap())
            nc.gpsimd.dma_start(out=battn, in_=battn_d.ap())
            nc.gpsimd.dma_start(out=tri01, in_=tri01_d.ap())
            nc.gpsimd.dma_start(out=onesrow, in_=onesrow_d.ap())

            wqkv = wpool.tile([128, CT, 3 * NPAIR, 128], BF16)
            wqkv_src = wqkv_d.ap().rearrange(
                "(ct p) (mt m) -> p ct mt m", p=128, m=128)
            wproj = wpool.tile([128, NPAIR, 2, 512], BF16)

            # ---- filler machinery ----
            fill_q = collections.deque()

            def pump(n=1):
                for _ in range(n):
                    while fill_q:
                        try:
                            next(fill_q[0])
                            break
                        except StopIteration:
                            fill_q.popleft()
                    else:
                        return

            def drain(gen):
                while any(g is gen for g in fill_q):
                    pump(1)

            ot_flip = [0]

            def qkv_gen(r, p, xT, qT, kT, vT, V2e):
                dests = (qT, kT, vT)
                for mt in range(3):
                    for th in range(2):
                        acc = ps.tile([128, 512], F32, tag="mm", bufs=2,
                                      name=f"acc_{r}_{p}_{mt}_{th}")
                        for ct in range(CT):
                            nc.tensor.matmul(
                                acc, wqkv[:, ct, p * 3 + mt, :],
                                xT[:, ct, ts(th, 512)],
                                start=(ct == 0), stop=(ct == CT - 1),
                                skip_group_check=True)
                            yield
                        col = p * 3 + mt
                        if th == 0:
                            nc.scalar.activation(
                                out=dests[mt][:, ts(th, 512)], in_=acc,
                                func=AF.Identity,
                                bias=battn[:, col:col + 1], scale=1.0)
                        else:
                            nc.vector.tensor_scalar_add(
                                out=dests[mt][:, ts(th, 512)], in0=acc,
                                scalar1=battn[:, col:col + 1])
                nc.gpsimd.memset(V2e[:, :, :, 64:65], 1.0)
                for kb in range(KB):
                    tp = ps.tile([128, 512], F32, tag="mm", bufs=2,
                                 name=f"tp_{r}_{p}_{kb}")
                    nc.tensor.matmul(tp[:, 0:128], vT[:, ts(kb, 128)], idb,
                                     start=True, stop=True,
                                     skip_group_check=True)
                    nc.vector.tensor_copy(
                        out=V2e[:, kb, :, 0:64],
                        in_=tp[:, 0:128].rearrange("p (h d) -> p h d", h=2))
                    yield

            def proj_gen(r, qc, yTs):
                for tt in range(4 * qc, 4 * qc + 4):
                    for oh in range(2):
                        pp = ps.tile([128, 512], F32, tag="mm", bufs=2,
                                     name=f"pp_{r}_{tt}_{oh}")
                        for pj in range(NPAIR):
                            nc.tensor.matmul(
                                pp, yTs[pj][:, ts(tt, 128)],
                                wproj[:, pj, oh, :],
                                start=(pj == 0), stop=(pj == NPAIR - 1),
                                skip_group_check=True)
                        ot = opool.tile([128, 512], BF16, tag="ot",
                                        name=f"ot_{r}_{tt}_{oh}")
                        if ot_flip[0] % 2 == 0:
                            nc.vector.tensor_copy(out=ot, in_=pp)
                        else:
                            nc.scalar.copy(out=ot, in_=pp)
                        ot_flip[0] += 1
                        nc.sync.dma_start(
                            out=out_d.ap()[ts(tt, 128), ts(oh, 512)],
                            in_=ot)
                        yield

            def alloc_qk(r, p):
                qT = qkpool.tile([128, 1024], BF16, tag="qT",
                                 name=f"qT_{r}_{p}")
                kT = qkpool.tile([128, 1024], BF16, tag="kT",
                                 name=f"kT_{r}_{p}")
                vT = qkpool.tile([128, 1024], BF16, tag="vT",
                                 name=f"vT_{r}_{p}")
                V2e = qkpool.tile([128, KB, 2, 65], BF16, tag="V2e",
                                  name=f"V2e_{r}_{p}")
                return qT, kT, vT, V2e

            def emit_x_load(xT):
                for ct in range(CT):
                    nc.sync.dma_start(
                        out=xT[:, ct, :],
                        in_=xT_d.ap()[ct * 128:(ct + 1) * 128, :])

            xT_tiles = {}
            qk_tiles = {}

            for r in range(reps):
                if r == 0:
                    xT = xpool.tile([128, CT, 1024], BF16, tag="xT",
                                    name="xT_0")
                    xT_tiles[0] = xT
                    for ct in range(CT):
                        # pair-0 weight slice first (Act HWDGE queue) so its
                        # qkv can stream against the incoming x chunks
                        nc.scalar.dma_start(out=wqkv[:, ct, 0:3, :],
                                            in_=wqkv_src[:, ct, 0:3, :])
                        nc.sync.dma_start(
                            out=xT[:, ct, :],
                            in_=xT_d.ap()[ct * 128:(ct + 1) * 128, :])
                    for ct in range(CT):
                        nc.gpsimd.dma_start(out=wqkv[:, ct, 3:12, :],
                                            in_=wqkv_src[:, ct, 3:12, :])
                    nc.gpsimd.dma_start(out=wproj, in_=wproj_d.ap().rearrange(
                        "(pr p) (oh n) -> p pr oh n", p=128, n=512))

                    # pair-0 qkv, ct-outer streaming with 6 live accumulators
                    qk_tiles[(0, 0)] = alloc_qk(0, 0)
                    qT0, kT0, vT0, V2e0 = qk_tiles[(0, 0)]
                    spq = ps.tile([128, 2, 512], F32, tag="sp", bufs=2,
                                  name="spq")
                    spk = ps.tile([128, 2, 512], F32, tag="sp", bufs=2,
                                  name="spk")
                    av0 = ps.tile([128, 512], F32, tag="mm", bufs=2,
                                  name="av0")
                    av1 = ps.tile([128, 512], F32, tag="mm", bufs=2,
                                  name="av1")
                    accs = {(0, 0): spq[:, 0, :], (0, 1): spq[:, 1, :],
                            (1, 0): spk[:, 0, :], (1, 1): spk[:, 1, :],
                            (2, 0): av0, (2, 1): av1}
                    for ct in range(CT):
                        for mt in range(3):
                            for th in range(2):
                                nc.tensor.matmul(
                                    accs[(mt, th)], wqkv[:, ct, mt, :],
                                    xT[:, ct, ts(th, 512)],
                                    start=(ct == 0), stop=(ct == CT - 1),
                                    skip_group_check=True)
                    dests0 = (qT0, kT0, vT0)
                    for mt in range(3):
                        for th in range(2):
                            nc.scalar.activation(
                                out=dests0[mt][:, ts(th, 512)],
                                in_=accs[(mt, th)], func=AF.Identity,
                                bias=battn[:, mt:mt + 1], scale=1.0)
                    nc.gpsimd.memset(V2e0[:, :, :, 64:65], 1.0)
                    for kb in range(KB):
                        tp = ps.tile([128, 512], F32, tag="mm", bufs=2,
                                     name=f"tp0_{kb}")
                        nc.tensor.matmul(tp[:, 0:128], vT0[:, ts(kb, 128)],
                                         idb, start=True, stop=True,
                                         skip_group_check=True)
                        nc.vector.tensor_copy(
                            out=V2e0[:, kb, :, 0:64],
                            in_=tp[:, 0:128].rearrange(
                                "p (h d) -> p h d", h=2))

                yTs = []
                for p in range(4):
                    qT, kT, vT, V2e = qk_tiles.pop((r, p))
                    yT = ypool.tile([128, 1024], BF16, tag="yT",
                                    name=f"yT_{r}_{p}")
                    yTs.append(yT)

                    if p == 0 and r + 1 < reps:
                        xn = xpool.tile([128, CT, 1024], BF16, tag="xT",
                                        name=f"xT_{r + 1}")
                        xT_tiles[r + 1] = xn
                        emit_x_load(xn)

                    if p < 3:
                        nxt = (r, p + 1)
                    elif r + 1 < reps:
                        nxt = (r + 1, 0)
                    else:
                        nxt = None
                    g = None
                    if nxt is not None:
                        qk_tiles[nxt] = alloc_qk(*nxt)
                        g = qkv_gen(nxt[0], nxt[1], xT_tiles[nxt[0]],
                                    *qk_tiles[nxt])
                        fill_q.append(g)

                    # attention for (r, p); V2e_cur must track THIS pair
                    V2e_local = V2e

                    def _pv(kb, qoff, pT, os_, kb_max, V2e_local=V2e_local):
                        nc.tensor.matmul(os_[:, 0, qoff:512],
                                         V2e_local[:, kb, 0, :],
                                         pT[:, 0, qoff:512], start=(kb == 0),
                                         stop=(kb == kb_max - 1),
                                         skip_group_check=True)
                        nc.tensor.matmul(os_[:, 1, qoff:512],
                                         V2e_local[:, kb, 1, :],
                                         pT[:, 1, qoff:512], start=(kb == 0),
                                         stop=(kb == kb_max - 1),
                                         skip_group_check=True)

                    for qc in range(QC):
                        os_ = ps.tile([65, 2, 512], F32, tag="os", bufs=1,
                                      name=f"os_{r}_{p}_{qc}")
                        kb_max = 4 * (qc + 1)
                        prevs = collections.deque()
                        for kb in range(kb_max):
                            qoff = max(0, kb * 128 - qc * 512)
                            diag = kb * 128 >= qc * 512
                            qs = slice(qc * 512 + qoff, (qc + 1) * 512)
                            sp = ps.tile([128, 2, 512], F32, tag="sp",
                                         bufs=2, name=f"sp_{r}_{p}_{qc}_{kb}")
                            nc.tensor.matmul(sp[:, 0, qoff:512],
                                             kT[0:64, ts(kb, 128)],
                                             qT[0:64, qs],
                                             start=True, stop=True,
                                             tile_position=(0, 0),
                                             skip_group_check=True)
                            nc.tensor.matmul(sp[:, 1, qoff:512],
                                             kT[64:128, ts(kb, 128)],
                                             qT[64:128, qs],
                                             start=True, stop=True,
                                             tile_position=(64, 0),
                                             skip_group_check=True)
                            # fillers BEFORE the pv so the in-order PE queue
                            # has work while exp(prev) completes; PV runs two
                            # steps behind its exp so the DVE causal-mask hop
                            # fits in the slack
                            pump(2 if kb < 2 else 1)
                            if len(prevs) >= 2:
                                _pv(*prevs.popleft())
                            pT = ppool.tile([128, 2, 512], BF16, tag="pT",
                                            name=f"pT_{r}_{p}_{qc}_{kb}")
                            nc.scalar.activation(out=pT[:, :, qoff:512],
                                                 in_=sp[:, :, qoff:512],
                                                 func=AF.Exp, scale=0.125)
                            if diag:
                                # causal mask: zero P above the diagonal on
                                # the DVE instead of -30000 adds on the PE
                                nc.vector.tensor_mul(
                                    pT[:, :, qoff:qoff + 128],
                                    pT[:, :, qoff:qoff + 128],
                                    tri01.unsqueeze(1).to_broadcast(
                                        [128, 2, 128]))
                            prevs.append((kb, qoff, pT, os_, kb_max))
                            pump(1)
                        while prevs:
                            _pv(*prevs.popleft())
                            pump(1)
                        # staggered per-half normalization: shortens the
                        # critical chain freeing os_ (bufs=1 WAR with the
                        # next qc's first PV). 1/denom broadcast via a tiny
                        # ones-row PE outer product into the mm tag; the DVE
                        # mul reads it straight from PSUM.
                        rec = rpool.tile([1, 2, 512], F32R, tag="rec",
                                         name=f"rec_{r}_{p}_{qc}")
                        with nc.allow_low_precision(reason="softmax den"):
                            nc.vector.reciprocal(
                                out=rec, in_=os_[64:65, :, :])
                        # moving O off PSUM frees the os bank early (bufs=1
                        # WAR with the next qc's first PV); halves split
                        # across Act and DVE so neither queue eats the full
                        # copy latency
                        ys = rpool.tile([64, 2, 512], F32, tag="ys",
                                        name=f"ys_{r}_{p}_{qc}")
                        nc.scalar.copy(out=ys[:, 0, :], in_=os_[0:64, 0, :])
                        nc.vector.tensor_copy(out=ys[:, 1, :],
                                              in_=os_[0:64, 1, :])
                        pump(1)
                        if BCAST_DMA:
                            # broadcast 1/denom to 64 partitions via a DRAM
                            # round-trip (stride-0 source) instead of PE
                            # outer-product matmuls
                            sel = bscr_d[(p * QC + qc) % 2]
                            nc.sync.dma_start(out=sel.ap(), in_=rec)
                            recb = rpool.tile([64, 2, 512], F32R, tag="recb",
                                              name=f"recb_{r}_{p}_{qc}")
                            nc.sync.dma_start(
                                out=recb,
                                in_=sel.ap().unsqueeze(0).to_broadcast(
                                    [64, 2, 512]))
                            for h in range(2):
                                nc.vector.tensor_mul(
                                    yT[h * 64:h * 64 + 64, ts(qc, 512)],
                                    ys[:, h, :], recb[:, h, :])
                        else:
                            for h in range(2):
                                bc = ps.tile([128, 512], F32, tag="mm",
                                             bufs=2,
                                             name=f"bc_{r}_{p}_{qc}_{h}")
                                nc.tensor.matmul(bc[0:64, :], onesrow,
                                                 rec[:, h, :], start=True,
                                                 stop=True,
                                                 skip_group_check=True)
                                nc.vector.tensor_mul(
                                    yT[h * 64:h * 64 + 64, ts(qc, 512)],
                                    ys[:, h, :], bc[0:64, :])
                        pump(3)
                        if p == 3:
                            fill_q.append(proj_gen(r, qc, yTs))

                    if g is not None:
                        drain(g)

            while fill_q:
                pump(1)

    _split_sync_waits(nc)
    return nc


def make_in_maps(x, w_attn, b_attn, w_proj):
    import ml_dtypes
    bf16 = ml_dtypes.bfloat16

    xT = np.asarray(x).reshape(B * T, C).T                      # [C, B*T]
    idb = np.eye(128, dtype=np.float32).astype(bf16)
    maskb = np.tril(np.full((128, 128), NEG, dtype=np.float32),
                    -1).astype(bf16)
    onesrow = np.ones((1, 64), dtype=np.float32)
    tri01 = np.triu(np.ones((128, 128), dtype=np.float32)).astype(bf16)

    xTb = [np.ascontiguousarray(xT[:, b * T:(b + 1) * T]).astype(bf16)
           for b in range(B)]
    in_maps = []
    for c in range(NCORES):
        bi, hg = divmod(c, 2)
        blocks, bias_cols, wp = [], [], []
        for pr in range(NPAIR):
            h0 = (hg * 8 + pr * 2) * D
            blocks += [w_attn[:, h0:h0 + 128],
                       w_attn[:, C + h0:C + h0 + 128],
                       w_attn[:, 2 * C + h0:2 * C + h0 + 128]]
            bias_cols += [b_attn[h0:h0 + 128],
                          b_attn[C + h0:C + h0 + 128],
                          b_attn[2 * C + h0:2 * C + h0 + 128]]
            wp.append(w_proj[h0:h0 + 128, :])
        wqkv = np.ascontiguousarray(
            np.concatenate(blocks, axis=1)).astype(bf16)            # [C,1536]
        battn = np.stack(bias_cols, axis=1).astype(np.float32)      # [128,12]
        wprojc = np.ascontiguousarray(
            np.concatenate(wp, axis=0)).astype(bf16)                # [512,C]
        in_maps.append({
            "xTb": xTb[bi], "wqkv": wqkv, "battn": battn, "wproj": wprojc,
            "idb": idb, "mask": maskb, "tri01": tri01,
            "onesrow": onesrow,
        })
    return in_maps


def kernel(x, w_attn, b_attn, w_proj, b_proj):
    x = np.asarray(x)
    w_attn = np.asarray(w_attn)
    b_attn = np.asarray(b_attn)
    w_proj = np.asarray(w_proj)
    b_proj = np.asarray(b_proj)

    if "nc" not in _nc_cache:
        _nc_cache["nc"] = build_nc()
    nc = _nc_cache["nc"]
    in_maps = make_in_maps(x, w_attn, b_attn, w_proj)

    res = run_bass_kernel_spmd(nc, in_maps, core_ids=list(range(NCORES)))
    out = np.empty((B, T, C), dtype=np.float32)
    for bi in range(B):
        out[bi] = res.results[2 * bi]["partial"].astype(np.float32)
        out[bi] += res.results[2 * bi + 1]["partial"].astype(np.float32)
        out[bi] += b_proj.astype(np.float32)
    return out

